# revision 24
# baseline (speedup 1.0000x reference)
"""Trainium2 Bass kernel for nn_BrainGTEnhanced (GATv2 + virtual node + per-graph
transformer + global gated pooling). Data-parallel: 16 graphs/core x 8 cores.

Residual stream is feature-major: x [128 feat (partitions), 4288 nodes (free)].
GNN edge pipeline runs in dst-sorted pad-64 slot order (268 runs/graph x 64
slots; slot 0 = self-loop, tail = dummy -> KILL column so exp() -> 0). The only
data-dependent op is one GPSIMD ap_gather per half-graph chunk, from an f32
table [xl | xl + la*w | KILL]. Per-dst softmax denominators and the weighted
aggregation are contiguous halving trees over the 64 pad slots.
"""
import numpy as np
import ml_dtypes

HID = 128; HEADS = 8; HDIM = 16; IN_DIM = 268
B = 128; NPG = 268; DEG = 32
NCORES = 8; G = B // NCORES               # 16 graphs per core
NC = G * NPG                              # 4288 nodes per core
EC = NC * DEG                             # 137216 edges per core
PAD = 64                                  # slots per node (1 self + indeg + pad)
QNPG = NPG // 4                           # 67 runs per quarter-graph chunk
QSL = QNPG * PAD                          # 4288 slots per chunk
NQ = 4 * G                                # 64 chunks per core
LN_EPS = 1e-5

_CACHE = {}


def _chunks(total, step):
    out, o = [], 0
    while o < total:
        out.append((o, min(step, total - o)))
        o += step
    return out


def _build():
    import concourse.bacc as bacc
    import concourse.mybir as mybir
    import concourse.tile as tile
    from concourse.masks import make_identity

    dt = mybir.dt
    bf = dt.bfloat16
    f32 = dt.float32
    f32r = dt.float32r
    AF = mybir.ActivationFunctionType
    ALU = mybir.AluOpType
    AX = mybir.AxisListType

    nc = bacc.Bacc("TRN2", target_bir_lowering=False, debug=False,
                   num_devices=NCORES)
    dram = {}

    def din(name, shape, dtype):
        dram[name] = nc.dram_tensor(name, shape, dtype, kind="ExternalInput")

    din("xT", [128, 3 * NC], f32)
    din("gidx", [128, (NQ * QSL) // 16], dt.int16)
    din("ea_row", [NQ, QSL], bf)
    din("vnemb", [HID, 1], f32)
    din("inWT", [128, 3 * HID], bf)
    din("inb", [HID, 1], f32)
    din("ing", [1, HID], bf)
    din("inbe", [HID, 1], f32)
    for l in range(2):
        din(f"WlT{l}", [HID, HID], bf)
        din(f"WrT{l}", [HID, HID], bf)
        din(f"bl{l}", [HID, 1], f32)
        din(f"br{l}", [HID, 1], f32)
        din(f"wvec{l}", [1, HID], bf)
        din(f"attR{l}", [HID, HID], bf)
        din(f"kill{l}", [HID, 1], f32)
        din(f"gbias{l}", [HID, 1], f32)
        din(f"lng{l}", [1, HID], bf)
        din(f"lnb{l}", [HID, 1], f32)
        din(f"e1WT{l}", [HID, 2 * HID], bf)
        din(f"e1b{l}", [128, 2], f32)
        din(f"e2WT{l}", [128, 2 * HID], bf)
        din(f"e2b{l}", [HID, 1], f32)
        din(f"d1WT{l}", [HID, 2 * HID], bf)
        din(f"d1b{l}", [128, 2], f32)
        din(f"d2WT{l}", [128, 2 * HID], bf)
        din(f"d2b{l}", [HID, 1], f32)
    for l in range(4):
        din(f"qTA{l}", [HID, 128], bf)
        din(f"qTB{l}", [HID, 128], bf)
        din(f"kTA{l}", [HID, 128], bf)
        din(f"kTB{l}", [HID, 128], bf)
        din(f"qbA{l}", [128, 1], f32)
        din(f"qbB{l}", [128, 1], f32)
        din(f"kbA{l}", [128, 1], f32)
        din(f"kbB{l}", [128, 1], f32)
        din(f"vTaug{l}", [HID, 136], bf)
        din(f"vbaug{l}", [1, 136], bf)
        din(f"woTA{l}", [128, HID], bf)
        din(f"woTB{l}", [128, HID], bf)
        din(f"wob{l}", [HID, 1], f32)
        din(f"w1T{l}", [HID, 4 * HID], bf)
        din(f"b1_{l}", [128, 4], f32)
        din(f"w2T{l}", [128, 4 * HID], bf)
        din(f"b2_{l}", [HID, 1], f32)
    din("R4c", [4, 128], bf)
    din("E4c", [128, 4], bf)
    din("gW1T", [HID, HID], bf)
    din("gb1", [HID, 1], f32)
    din("gW2T", [HID, 1], bf)
    din("gb2", [1, 1], f32)

    xp_o = nc.dram_tensor("xp_o", [HID, G], f32, kind="ExternalOutput")
    gws_o = nc.dram_tensor("gws_o", [1, 1], f32, kind="ExternalOutput")
    vn_o = nc.dram_tensor("vn_o", [HID, G], f32, kind="ExternalOutput")

    with tile.TileContext(nc) as tc:
      with (
        nc.allow_low_precision(reason="bf16 segment-sum trees are intentional"),
        tc.tile_pool(name="const", bufs=1) as cpool,
        tc.tile_pool(name="stream", bufs=1) as pool,
        tc.tile_pool(name="sm", bufs=1) as spool,
        tc.tile_pool(name="ps", bufs=3, space="PSUM") as pp,
        tc.tile_pool(name="ps15", bufs=1, space="PSUM") as pp15,
        tc.tile_pool(name="psr", bufs=1, space="PSUM") as ppr,
      ):
        identf = cpool.tile([128, 128], f32)
        make_identity(nc, identf[:])
        identb = cpool.tile([128, 128], bf)
        make_identity(nc, identb[:])
        ones_row = cpool.tile([1, 512], bf)
        nc.vector.memset(ones_row[:], 1.0)
        onesb_row = cpool.tile([1, 128], bf)
        nc.vector.memset(onesb_row[:], 1.0)
        o128 = cpool.tile([128, 1], bf)
        nc.vector.memset(o128[:], 1.0 / 128.0)
        zb128 = cpool.tile([128, 1], f32)
        nc.vector.memset(zb128[:], 0.0)
        eps1 = cpool.tile([1, 1], f32)
        nc.vector.memset(eps1[:], LN_EPS)

        def load(name, pl=None):
            t = (pl or cpool).tile(list(dram[name].shape), dram[name].dtype,
                                   tag=name)
            nc.sync.dma_start(t[:], dram[name].ap())
            return t

        _skip = {"xT", "gidx", "ea_row"}
        _tf_pref = ("qT", "qb", "kT", "kb", "vT", "vb", "woT", "wob",
                    "w1T", "b1_", "w2T", "b2_")
        P = {k: load(k) for k in dram
             if k not in _skip and not k.startswith(_tf_pref)}

        xf = pool.tile([128, NC], f32)     # residual stream master
        xb = pool.tile([128, NC], bf)      # bf16 shadow
        vn = pool.tile([128, G], f32)
        nc.vector.tensor_copy(vn[:], P["vnemb"][:].to_broadcast([128, G]))

        def ln_feat(srcf, srcb, gain_row, bias_col, relu):
            """x <- relu?(LN_feat(src)) written into xf (f32) and xb (bf16).
            srcf/srcb: callables (o, w) -> f32 / bf16 chunk APs."""
            mu_t = pool.tile([1, NC], f32, tag="ln_mu")
            s2_t = pool.tile([1, NC], f32, tag="ln_s2")
            mu = mu_t[:]
            s2 = s2_t[:]
            for o, w in _chunks(NC, 512):
                sq = spool.tile([128, 512], bf, tag="ln_sq")
                mps = pp.tile([1, 512], f32, tag="ps")
                nc.tensor.matmul(mps[:, :w], o128[:], srcb(o, w),
                                 start=True, stop=True)
                nc.vector.tensor_copy(mu_t[:, o:o + w], mps[:, :w])
                nc.scalar.activation(sq[:, :w], srcb(o, w), AF.Square, bias=zb128[:])
                mps2 = pp.tile([1, 512], f32, tag="ps")
                nc.tensor.matmul(mps2[:, :w], o128[:], sq[:, :w],
                                 start=True, stop=True)
                nc.vector.tensor_copy(s2_t[:, o:o + w], mps2[:, :w])
            var = s2
            tmp = spool.tile([1, NC], bf, tag="ln_tmp")
            nc.vector.tensor_tensor(tmp[:], mu, mu, op=ALU.mult)
            nc.vector.tensor_tensor(var, s2, tmp[:], op=ALU.subtract)
            nc.scalar.activation(var, var, AF.Sqrt, bias=eps1[:])
            inv = var
            nc.vector.reciprocal(inv, var)
            mub_t = pool.tile([1, NC], bf, tag="ln_mub")
            invb_t = pool.tile([1, NC], bf, tag="ln_invb")
            mub = mub_t[:]
            invb = invb_t[:]
            nc.vector.tensor_copy(mub, mu)
            nc.vector.tensor_copy(invb, inv)
            for o, w in _chunks(NC, 512):
                m128 = pp.tile([128, 512], f32, tag="ps")
                nc.tensor.matmul(m128[:, :w], onesb_row[:], mub[0:1, o:o + w],
                                 start=True, stop=True)
                g128 = pp.tile([128, 512], f32, tag="ps")
                nc.tensor.matmul(g128[:, :w], gain_row[:], invb[0:1, o:o + w],
                                 start=True, stop=True)
                t = spool.tile([128, 512], f32, tag="ln_t")
                nc.vector.tensor_tensor(t[:, :w], srcf(o, w), m128[:, :w],
                                        op=ALU.subtract)
                nc.vector.tensor_tensor(t[:, :w], t[:, :w], g128[:, :w],
                                        op=ALU.mult)
                fn = AF.Relu if relu else AF.Identity
                nc.scalar.activation(xf[:, o:o + w], t[:, :w], fn,
                                     bias=bias_col[:])
                nc.vector.tensor_copy(xb[:, o:o + w], xf[:, o:o + w])

        # ---------------- phase 1: input projection + LN + relu ------------
        with tc.tile_pool(name="ph1", bufs=1) as p1:
            xTb = p1.tile([128, 3 * NC], bf)
            nc.gpsimd.dma_start(xTb[:], dram["xT"].ap())
            y0 = p1.tile([128, NC], f32)
            y0b = p1.tile([128, NC], bf)
            for o, w in _chunks(NC, 512):
                yps = pp.tile([128, 512], f32, tag="ps")
                for kt in range(3):
                    nc.tensor.matmul(yps[:, :w],
                                     P["inWT"][:, kt * HID:(kt + 1) * HID],
                                     xTb[:, kt * NC + o:kt * NC + o + w],
                                     start=(kt == 0), stop=(kt == 2))
                nc.scalar.activation(y0[:, o:o + w], yps[:, :w], AF.Identity,
                                     bias=P["inb"][:])
                nc.vector.tensor_copy(y0b[:, o:o + w], y0[:, o:o + w])
            ln_feat(lambda o, w: y0[:, o:o + w], lambda o, w: y0b[:, o:o + w],
                    P["ing"], P["inbe"], relu=True)

        # ---------------- phase 2: GNN layers ------------------------------
        for l in range(2):
          with tc.tile_pool(name=f"gnn{l}", bufs=1) as gp, \
               tc.tile_pool(name=f"ge{l}", bufs=1) as gep:
            xl = gp.tile([128, NC], bf)
            xr = gp.tile([128, NC], bf)
            table = gp.tile([128, NC + 1], f32)
            for o, w in _chunks(NC, 512):
                ps = pp.tile([128, 512], f32, tag="ps")
                nc.tensor.matmul(ps[:, :w], P[f"WlT{l}"][:], xb[:, o:o + w],
                                 start=True, stop=True)
                nc.scalar.activation(table[:, o:o + w], ps[:, :w], AF.Identity,
                                     bias=P[f"bl{l}"][:])
                nc.vector.tensor_copy(xl[:, o:o + w], table[:, o:o + w])
                ps2 = pp.tile([128, 512], f32, tag="ps")
                nc.tensor.matmul(ps2[:, :w], P[f"WrT{l}"][:], xb[:, o:o + w],
                                 start=True, stop=True)
                nc.scalar.activation(xr[:, o:o + w], ps2[:, :w], AF.Identity,
                                     bias=P[f"br{l}"][:])
            nc.vector.tensor_copy(table[:, NC:NC + 1], P[f"kill{l}"][:])

            num = gp.tile([128, NC], bf)
            den = gp.tile([128, NC], bf)

            for h in range(NQ):
                node0 = h * QNPG
                so = h * QSL
                gidx_t = gep.tile([128, QSL // 16], dt.int16, tag="gidx")
                nc.sync.dma_start(gidx_t[:],
                                  dram["gidx"].ap()[:, so // 16:(so + QSL) // 16])
                ea_t = gep.tile([1, QSL], bf, tag="ea_t")
                nc.sync.dma_start(ea_t[:], dram["ea_row"].ap()[h:h + 1, :])
                xlg = gep.tile([128, QSL], f32, tag="xlg")
                nc.gpsimd.ap_gather(
                    xlg[:].unsqueeze(2), table[:].unsqueeze(2), gidx_t[:],
                    channels=128, num_elems=NC + 1, d=1, num_idxs=QSL)
                lr = gep.tile([128, QSL], bf, tag="lrv")
                for co, cw in _chunks(QSL, 512):
                    ups = pp.tile([128, 512], f32, tag="ps")
                    nd0 = node0 + co // PAD
                    ndw = cw // PAD
                    nc.tensor.matmul(ups[:, :cw], identb[:],
                                     xr[:, nd0:nd0 + ndw].unsqueeze(2)
                                     .to_broadcast([128, ndw, PAD]),
                                     start=True, stop=False)
                    nc.tensor.matmul(ups[:, :cw], P[f"wvec{l}"][:],
                                     ea_t[:, co:co + cw],
                                     start=False, stop=True)
                    nc.vector.tensor_tensor(lr[:, co:co + cw],
                                            xlg[:, co:co + cw], ups[:, :cw],
                                            op=ALU.add)
                nc.scalar.activation(lr[:], lr[:], AF.Lrelu, bias=zb128[:],
                                     alpha=0.2)
                beta = gep.tile([128, QSL], bf, tag="beta")
                for co, cw in _chunks(QSL, 1536):
                    lps = pp15.tile([128, 1536], f32, tag="ps1536")
                    for so2, sw in _chunks(cw, 512):
                        nc.tensor.matmul(lps[:, so2:so2 + sw],
                                         P[f"attR{l}"][:],
                                         lr[:, co + so2:co + so2 + sw],
                                         start=True, stop=True)
                    nc.scalar.activation(beta[:, co:co + cw], lps[:, :cw],
                                         AF.Exp, bias=zb128[:])
                V = gep.tile([128, QSL], bf, tag="lrv")
                nc.vector.tensor_tensor(V[:], beta[:], xlg[:], op=ALU.mult)

                def tree(src, dst_slice):
                    a = src[:].rearrange("p (n j) -> p n j", j=PAD)
                    width = PAD
                    while width > 1:
                        hw = width // 2
                        nc.vector.tensor_tensor(
                            a[:, :, 0:hw], a[:, :, 0:hw], a[:, :, hw:width],
                            op=ALU.add)
                        width = hw
                    nc.vector.tensor_copy(dst_slice,
                                          a[:, :, 0:1].rearrange("p n j -> p (n j)"))

                tree(V, num[:, node0:node0 + QNPG])
                tree(beta, den[:, node0:node0 + QNPG])

            # x <- LN(num/den + gbias + x); xb reused as bf16 shadow of h+x
            for o, w in _chunks(NC, 512):
                dr = spool.tile([128, 512], bf, tag="gatrec")
                nc.vector.reciprocal(dr[:, :w], den[:, o:o + w])
                q = spool.tile([128, 512], bf, tag="gatdiv")
                nc.vector.tensor_tensor(q[:, :w], num[:, o:o + w],
                                        dr[:, :w], op=ALU.mult)
                t2 = spool.tile([128, 512], f32, tag="gatb")
                nc.scalar.activation(t2[:, :w], q[:, :w], AF.Identity,
                                     bias=P[f"gbias{l}"][:])
                nc.vector.tensor_tensor(xf[:, o:o + w], t2[:, :w],
                                        xf[:, o:o + w], op=ALU.add)
                nc.vector.tensor_copy(xb[:, o:o + w], xf[:, o:o + w])
            ln_feat(lambda o, w: xf[:, o:o + w], lambda o, w: xb[:, o:o + w],
                    P[f"lng{l}"], P[f"lnb{l}"], relu=False)

            # virtual node
            vnu = spool.tile([128, G], bf, tag="vnu")
            vnuf = spool.tile([128, G], f32, tag="vnuf")
            for g in range(G):
                nc.vector.tensor_reduce(vnuf[:, g:g + 1],
                                        xf[:, g * NPG:(g + 1) * NPG],
                                        axis=AX.X, op=ALU.add)
            nc.vector.tensor_scalar(vnu[:], vnuf[:], 1.0 / NPG, None,
                                    op0=ALU.mult)

            def mlp2(in_bf, W1, b1, W2, b2, out_f32, add_to):
                hid_t = spool.tile([128, 2 * G], bf, tag="mlp_h")
                for mt in range(2):
                    ps = pp.tile([128, G], f32, tag="ps")
                    nc.tensor.matmul(ps[:], P[W1][:, mt * 128:(mt + 1) * 128],
                                     in_bf[:], start=True, stop=True)
                    nc.scalar.activation(hid_t[:, mt * G:(mt + 1) * G], ps[:],
                                         AF.Relu, bias=P[b1][:, mt:mt + 1])
                ps2 = pp.tile([128, G], f32, tag="ps")
                for kt in range(2):
                    nc.tensor.matmul(ps2[:],
                                     P[W2][:, kt * 128:(kt + 1) * 128],
                                     hid_t[:, kt * G:(kt + 1) * G],
                                     start=(kt == 0), stop=(kt == 1))
                t = spool.tile([128, G], f32, tag="mlp_t")
                nc.scalar.activation(t[:], ps2[:], AF.Identity, bias=P[b2][:])
                if add_to is None:
                    nc.vector.tensor_copy(out_f32[:], t[:])
                else:
                    nc.vector.tensor_tensor(out_f32[:], t[:], add_to[:],
                                            op=ALU.add)

            mlp2(vnu, f"e1WT{l}", f"e1b{l}", f"e2WT{l}", f"e2b{l}", vn, vn)
            vnb = spool.tile([128, G], bf, tag="vnb")
            nc.vector.tensor_copy(vnb[:], vn[:])
            dec = spool.tile([128, G], f32, tag="dec")
            mlp2(vnb, f"d1WT{l}", f"d1b{l}", f"d2WT{l}", f"d2b{l}", dec, None)
            for g in range(G):
                nc.vector.tensor_tensor(
                    xf[:, g * NPG:(g + 1) * NPG], xf[:, g * NPG:(g + 1) * NPG],
                    dec[:, g:g + 1].to_broadcast([128, NPG]), op=ALU.add)
            for o, w in _chunks(NC, 512):
                nc.vector.tensor_copy(xb[:, o:o + w], xf[:, o:o + w])

        # ---------------- phase 3: transformer layers ----------------------
        for l in range(4):
          with tc.tile_pool(name=f"tf{l}", bufs=1) as tp:
            for _k in list(dram):
                if _k.startswith(("qT", "qb", "kT", "kb", "vT", "vb", "woT",
                                  "wob", "w1T", "b1_", "w2T", "b2_"))                         and _k.endswith(str(l)):
                    P[_k] = load(_k, tp)

            def graphnorm(dst):
                mu = spool.tile([128, G], f32, tag="gn_mu")
                sg = spool.tile([128, G], f32, tag="gn_sg")
                for g in range(G):
                    xg = xf[:, g * NPG:(g + 1) * NPG]
                    nc.vector.tensor_reduce(mu[:, g:g + 1], xg, axis=AX.X,
                                            op=ALU.add)
                    sq2 = spool.tile([128, NPG], f32, tag="gn_sq")
                    nc.scalar.activation(sq2[:], xg, AF.Square,
                                         bias=zb128[:],
                                         accum_out=sg[:, g:g + 1])
                nc.vector.tensor_scalar(mu[:], mu[:], 1.0 / NPG, None,
                                        op0=ALU.mult)
                musq = spool.tile([128, G], f32, tag="gn_m2")
                nc.vector.tensor_tensor(musq[:], mu[:], mu[:], op=ALU.mult)
                nc.vector.tensor_scalar(musq[:], musq[:], float(NPG), None,
                                        op0=ALU.mult)
                nc.vector.tensor_tensor(sg[:], sg[:], musq[:], op=ALU.subtract)
                nc.vector.tensor_scalar(sg[:], sg[:], 1.0 / (NPG - 1), None,
                                        op0=ALU.mult)
                nc.scalar.activation(sg[:], sg[:], AF.Sqrt, bias=zb128[:])
                nc.vector.tensor_scalar(sg[:], sg[:], LN_EPS, None, op0=ALU.add)
                inv = spool.tile([128, G], f32, tag="gn_iv")
                nc.vector.reciprocal(inv[:], sg[:])
                for g in range(G):
                    nc.vector.tensor_scalar(
                        dst[:, g * NPG:(g + 1) * NPG],
                        xf[:, g * NPG:(g + 1) * NPG],
                        mu[:, g:g + 1], inv[:, g:g + 1],
                        op0=ALU.subtract, op1=ALU.mult)

            xn = tp.tile([128, NC], bf, tag="gn_x")
            graphnorm(xn)
            qA = tp.tile([128, NC], bf)
            qB = tp.tile([128, NC], bf)
            kA = tp.tile([128, NC], bf)
            kB = tp.tile([128, NC], bf)
            for o, w in _chunks(NC, 512):
                for nm, wt, bias in ((qA, f"qTA{l}", f"qbA{l}"),
                                     (qB, f"qTB{l}", f"qbB{l}"),
                                     (kA, f"kTA{l}", f"kbA{l}"),
                                     (kB, f"kTB{l}", f"kbB{l}")):
                    ps = pp.tile([128, 512], f32, tag="ps")
                    nc.tensor.matmul(ps[:, :w], P[wt][:], xn[:, o:o + w],
                                     start=True, stop=True)
                    nc.scalar.activation(nm[:, o:o + w], ps[:, :w],
                                         AF.Identity, bias=P[bias][:])

            scale = 1.0 / float(np.sqrt(HDIM))
            ktiles = _chunks(NPG, 128)
            for g in range(G):
                n0 = g * NPG
                # v node-major (augmented with ones col per head)
                vnm = tp.tile([128, 3 * 136], bf, tag="vnm")
                for kt, (ko, kw) in enumerate(ktiles):
                    psv = pp.tile([128, 136], f32, tag="ps")
                    nc.tensor.matmul(psv[:kw, :], xn[:, n0 + ko:n0 + ko + kw],
                                     P[f"vTaug{l}"][:], start=True, stop=False)
                    nc.tensor.matmul(psv[:kw, :], ones_row[:, :kw],
                                     P[f"vbaug{l}"][:], start=False, stop=True)
                    nc.vector.tensor_copy(vnm[:kw, kt * 136:(kt + 1) * 136],
                                          psv[:kw, :])
                beta_t = tp.tile([128, 24 * NPG], bf, tag="mha_beta")
                for kt, (ko, kw) in enumerate(ktiles):
                    for hp, (kmat, qmat) in enumerate(((kA, qA), (kB, qB))):
                        for j in range(4):
                            sps = pp.tile([128, NPG], f32, tag="ps")
                            nc.tensor.matmul(
                                sps[:kw, :],
                                kmat[32 * j:32 * j + 32,
                                     n0 + ko:n0 + ko + kw],
                                qmat[32 * j:32 * j + 32, n0:n0 + NPG],
                                start=True, stop=True,
                                tile_position=(32 * j, 0))
                            hh = hp * 4 + j
                            slot = (kt * 8 + hh) * NPG
                            nc.scalar.activation(
                                beta_t[:, slot:slot + NPG][:kw, :],
                                sps[:kw, :], AF.Exp, bias=zb128[:kw, :],
                                scale=scale)
                oA = ppr.tile([128, NPG], f32, tag="oA")
                oB = ppr.tile([128, NPG], f32, tag="oB")
                for kt, (ko, kw) in enumerate(ktiles):
                    for hh in range(8):
                        slot = (kt * 8 + hh) * NPG
                        dst = oA if hh < 4 else oB
                        j = hh % 4
                        nc.tensor.matmul(
                            dst[32 * j:32 * j + 17, :],
                            vnm[:kw, kt * 136 + 17 * hh:kt * 136 + 17 * hh + 17],
                            beta_t[:kw, slot:slot + NPG],
                            start=(kt == 0), stop=(kt == 2),
                            skip_group_check=True,
                            tile_position=(0, 32 * j))
                pso = pp.tile([128, NPG], f32, tag="ps")
                for di, dst in enumerate((oA, oB)):
                    # extract the 4 denominator rows (32j+16) via E4 matmul
                    dsb = spool.tile([128, NPG], bf, tag="osb_raw")
                    nc.vector.tensor_copy(dsb[:], dst[:])
                    dps = pp.tile([4, NPG], f32, tag="ps")
                    nc.tensor.matmul(dps[:], P["E4c"][:], dsb[:], start=True,
                                     stop=True)
                    rec = spool.tile([4, NPG], bf, tag="orec")
                    nc.vector.reciprocal(rec[:], dps[:])
                    rep = pp.tile([128, NPG], f32, tag="ps")
                    nc.tensor.matmul(rep[:], P["R4c"][:], rec[:], start=True,
                                     stop=True)
                    repb = spool.tile([128, NPG], bf, tag="repb")
                    nc.vector.tensor_copy(repb[:], rep[:])
                    osb = spool.tile([128, NPG], bf, tag="osb")
                    nc.vector.tensor_tensor(osb[:], dst[:], repb[:],
                                            op=ALU.mult)
                    nc.tensor.matmul(pso[:],
                                     P[f"woTA{l}"][:] if di == 0
                                     else P[f"woTB{l}"][:], osb[:],
                                     start=(di == 0), stop=(di == 1))
                t3 = spool.tile([128, NPG], f32, tag="oproj_t")
                nc.scalar.activation(t3[:], pso[:], AF.Identity,
                                     bias=P[f"wob{l}"][:])
                nc.vector.tensor_tensor(xf[:, n0:n0 + NPG], xf[:, n0:n0 + NPG],
                                        t3[:], op=ALU.add)

            xn2 = tp.tile([128, NC], bf, tag="gn_x2")
            graphnorm(xn2)
            for o, w in _chunks(NC, 512):
                hsb = tp.tile([128, 2048], bf, tag="ffn_h")
                for mt in range(4):
                    ps = pp.tile([128, 512], f32, tag="ps")
                    nc.tensor.matmul(ps[:, :w],
                                     P[f"w1T{l}"][:, mt * 128:(mt + 1) * 128],
                                     xn2[:, o:o + w], start=True, stop=True)
                    nc.scalar.activation(
                        hsb[:, mt * 512:mt * 512 + w], ps[:, :w], AF.Gelu,
                        bias=P[f"b1_{l}"][:, mt:mt + 1])
                ps2 = pp.tile([128, 512], f32, tag="ps")
                for kt in range(4):
                    nc.tensor.matmul(ps2[:, :w],
                                     P[f"w2T{l}"][:, kt * 128:(kt + 1) * 128],
                                     hsb[:, kt * 512:kt * 512 + w],
                                     start=(kt == 0), stop=(kt == 3))
                t4 = spool.tile([128, 512], f32, tag="ffn_t")
                nc.scalar.activation(t4[:, :w], ps2[:, :w], AF.Identity,
                                     bias=P[f"b2_{l}"][:])
                nc.vector.tensor_tensor(xf[:, o:o + w], xf[:, o:o + w],
                                        t4[:, :w], op=ALU.add)

        # ---------------- phase 4: gated pooling ---------------------------
        with tc.tile_pool(name="ph4", bufs=1) as p4:
            xbf = p4.tile([128, NC], bf)
            for o, w in _chunks(NC, 512):
                nc.vector.tensor_copy(xbf[:, o:o + w], xf[:, o:o + w])
            gate = p4.tile([1, NC], f32)
            for o, w in _chunks(NC, 512):
                ps = pp.tile([128, 512], f32, tag="ps")
                nc.tensor.matmul(ps[:, :w], P["gW1T"][:], xbf[:, o:o + w],
                                 start=True, stop=True)
                th = spool.tile([128, 512], bf, tag="g_tanh")
                nc.scalar.activation(th[:, :w], ps[:, :w], AF.Tanh,
                                     bias=P["gb1"][:])
                ps2 = pp.tile([1, 512], f32, tag="ps")
                nc.tensor.matmul(ps2[:, :w], P["gW2T"][:], th[:, :w],
                                 start=True, stop=True)
                nc.vector.tensor_copy(gate[:, o:o + w], ps2[:, :w])
            gwu = p4.tile([1, NC], f32)
            nc.scalar.activation(gwu[:], gate[:], AF.Exp, bias=P["gb2"][:])
            gwub = p4.tile([1, NC], bf)
            nc.vector.tensor_copy(gwub[:], gwu[:])
            gws = spool.tile([1, 1], f32, tag="gws")
            nc.vector.tensor_reduce(gws[:], gwu[:], axis=AX.X, op=ALU.add)
            nc.sync.dma_start(gws_o.ap(), gws[:])
            gwur = p4.tile([128, NC], f32)
            for o, w in _chunks(NC, 512):
                ps = pp.tile([128, 512], f32, tag="ps")
                nc.tensor.matmul(ps[:, :w], onesb_row[:], gwub[:, o:o + w],
                                 start=True, stop=True)
                nc.vector.tensor_tensor(gwur[:, o:o + w], xf[:, o:o + w],
                                        ps[:, :w], op=ALU.mult)
            xp = spool.tile([128, G], f32, tag="xp")
            for g in range(G):
                nc.vector.tensor_reduce(xp[:, g:g + 1],
                                        gwur[:, g * NPG:(g + 1) * NPG],
                                        axis=AX.X, op=ALU.add)
            nc.sync.dma_start(xp_o.ap(), xp[:])
            nc.sync.dma_start(vn_o.ap(), vn[:])

    nc.compile()
    return nc


def _prep_core(x, ea, src, dst):
    ins = {}
    xTp = np.zeros((128, 3, NC), np.float32)
    xt = x.T.astype(np.float32)                      # [268, NC]
    for kt in range(3):
        kw = min(128, IN_DIM - kt * 128)
        xTp[:kw, kt, :] = xt[kt * 128:kt * 128 + kw]
    ins["xT"] = np.ascontiguousarray(xTp.reshape(128, 3 * NC))
    order = np.argsort(dst, kind="stable")
    dsts = dst[order]
    srcs = src[order]
    eas = ea[order]
    indeg = np.bincount(dst, minlength=NC)
    assert indeg.max() + 1 <= PAD, f"max indeg {indeg.max()}"
    gidx = np.full(NQ * QSL, NC, np.int64)
    ea_slot = np.zeros(NQ * QSL, np.float32)
    starts = np.zeros(NC + 1, np.int64)
    np.cumsum(indeg, out=starts[1:])
    la = np.zeros(NC, np.float32)
    np.add.at(la, dst, ea)
    la = la / np.maximum(indeg, 1)
    gidx[np.arange(NC, dtype=np.int64) * PAD] = np.arange(NC)
    ea_slot[np.arange(NC, dtype=np.int64) * PAD] = la
    rank = np.arange(len(dsts)) - starts[dsts]
    pos = dsts * PAD + 1 + rank
    gidx[pos] = srcs
    ea_slot[pos] = eas
    gw = gidx.reshape(NQ, QSL // 16, 16)
    wrapped = np.transpose(gw, (0, 2, 1))               # [NQ, 16, QSL//16]
    full = np.tile(wrapped, (1, 8, 1))                  # [NQ, 128, QSL//16]
    ins["gidx"] = np.ascontiguousarray(
        np.concatenate(list(full), axis=1)).astype(np.int16)
    ins["ea_row"] = ea_slot.reshape(NQ, QSL).astype(ml_dtypes.bfloat16)
    return ins


def _prep_params(p):
    bfd = ml_dtypes.bfloat16
    o = {}

    def BF(a):
        return np.ascontiguousarray(np.asarray(a, np.float32)).astype(bfd)

    def F(a):
        return np.ascontiguousarray(np.asarray(a, np.float32))

    o["vnemb"] = F(np.asarray(p["vn_emb"]).reshape(HID, 1))
    inwt = np.zeros((128, 3, HID), np.float32)
    iw = np.asarray(p["inW"], np.float32).T          # [268, 128]
    for kt in range(3):
        kw = min(128, IN_DIM - kt * 128)
        inwt[:kw, kt, :] = iw[kt * 128:kt * 128 + kw]
    o["inWT"] = BF(inwt.reshape(128, 3 * HID))
    o["inb"] = F(np.asarray(p["inb"]).reshape(HID, 1))
    o["ing"] = BF(np.asarray(p["ing"]).reshape(1, HID))
    o["inbe"] = F(np.asarray(p["inbe"]).reshape(HID, 1))
    for l, gp in enumerate(p["gnn"]):
        o[f"WlT{l}"] = BF(np.asarray(gp["Wl"]).T)
        o[f"WrT{l}"] = BF(np.asarray(gp["Wr"]).T)
        o[f"bl{l}"] = F(np.asarray(gp["bl"]).reshape(HID, 1))
        o[f"br{l}"] = F(np.asarray(gp["br"]).reshape(HID, 1))
        w = np.asarray(gp["We"], np.float32)[:, 0]
        o[f"wvec{l}"] = BF(w.reshape(1, HID))
        att = np.asarray(gp["att"], np.float32)
        attR = np.zeros((HID, HID), np.float32)
        for m in range(HID):
            hh = m // 16
            attR[16 * hh:16 * hh + 16, m] = att[hh]
        o[f"attR{l}"] = BF(attR)
        o[f"kill{l}"] = F(
            (-1e4 * np.where(att.reshape(-1) >= 0, 1.0, -1.0)).reshape(HID, 1))
        o[f"gbias{l}"] = F(np.asarray(gp["bias"]).reshape(HID, 1))
        o[f"lng{l}"] = BF(np.asarray(gp["ln_g"]).reshape(1, HID))
        o[f"lnb{l}"] = F(np.asarray(gp["ln_b"]).reshape(HID, 1))
    for l, vp in enumerate(p["vn"]):
        o[f"e1WT{l}"] = BF(np.asarray(vp["e1W"]).T)
        o[f"e1b{l}"] = F(np.asarray(vp["e1b"]).reshape(2, 128).T)
        e2t = np.asarray(vp["e2W"], np.float32).T    # [256, 128]
        o[f"e2WT{l}"] = BF(np.concatenate([e2t[:128], e2t[128:]], axis=1))
        o[f"e2b{l}"] = F(np.asarray(vp["e2b"]).reshape(HID, 1))
        o[f"d1WT{l}"] = BF(np.asarray(vp["d1W"]).T)
        o[f"d1b{l}"] = F(np.asarray(vp["d1b"]).reshape(2, 128).T)
        d2t = np.asarray(vp["d2W"], np.float32).T
        o[f"d2WT{l}"] = BF(np.concatenate([d2t[:128], d2t[128:]], axis=1))
        o[f"d2b{l}"] = F(np.asarray(vp["d2b"]).reshape(HID, 1))
    for l, tp in enumerate(p["tf"]):
        Win = np.asarray(tp["Win"], np.float32)
        bin_ = np.asarray(tp["bin"], np.float32)
        gw = np.asarray(tp["gn1w"], np.float32).reshape(1, HID)
        gb = np.asarray(tp["gn1b"], np.float32).reshape(1, HID)
        Wq, Wk, Wv = Win[:HID], Win[HID:2 * HID], Win[2 * HID:]
        bq, bk, bv = bin_[:HID], bin_[HID:2 * HID], bin_[2 * HID:]
        Wq_f = Wq * gw; bq_f = bq + (Wq @ gb.T)[:, 0]
        Wk_f = Wk * gw; bk_f = bk + (Wk @ gb.T)[:, 0]
        Wv_f = Wv * gw; bv_f = bv + (Wv @ gb.T)[:, 0]

        def padheads(W, bias, lo):
            Wp = np.zeros((128, HID), np.float32)
            bp = np.zeros((128, 1), np.float32)
            for j in range(4):
                hh = lo + j
                Wp[32 * j:32 * j + 16] = W[16 * hh:16 * hh + 16]
                bp[32 * j:32 * j + 16, 0] = bias[16 * hh:16 * hh + 16]
            return BF(Wp.T), F(bp)

        o[f"qTA{l}"], o[f"qbA{l}"] = padheads(Wq_f, bq_f, 0)
        o[f"qTB{l}"], o[f"qbB{l}"] = padheads(Wq_f, bq_f, 4)
        o[f"kTA{l}"], o[f"kbA{l}"] = padheads(Wk_f, bk_f, 0)
        o[f"kTB{l}"], o[f"kbB{l}"] = padheads(Wk_f, bk_f, 4)
        Wva = np.zeros((136, HID), np.float32)
        bva = np.zeros(136, np.float32)
        for hh in range(8):
            Wva[17 * hh:17 * hh + 16] = Wv_f[16 * hh:16 * hh + 16]
            bva[17 * hh:17 * hh + 16] = bv_f[16 * hh:16 * hh + 16]
            bva[17 * hh + 16] = 1.0
        o[f"vTaug{l}"] = BF(Wva.T)
        o[f"vbaug{l}"] = BF(bva.reshape(1, 136))
        Wout = np.asarray(tp["Wout"], np.float32)
        woA = np.zeros((128, HID), np.float32)
        woB = np.zeros((128, HID), np.float32)
        for j in range(4):
            woA[32 * j:32 * j + 16] = Wout[:, 16 * j:16 * j + 16].T
            woB[32 * j:32 * j + 16] = Wout[:, 16 * (j + 4):16 * (j + 4) + 16].T
        o[f"woTA{l}"] = BF(woA)
        o[f"woTB{l}"] = BF(woB)
        o[f"wob{l}"] = F(np.asarray(tp["bout"]).reshape(HID, 1))
        g2w = np.asarray(tp["gn2w"], np.float32).reshape(1, HID)
        g2b = np.asarray(tp["gn2b"], np.float32).reshape(1, HID)
        W1 = np.asarray(tp["W1"], np.float32)
        b1 = np.asarray(tp["b1"], np.float32)
        o[f"w1T{l}"] = BF((W1 * g2w).T)
        o[f"b1_{l}"] = F((b1 + (W1 @ g2b.T)[:, 0]).reshape(4, 128).T)
        w2t = np.asarray(tp["W2"], np.float32).T     # [512, 128]
        o[f"w2T{l}"] = BF(np.concatenate([w2t[128 * i:128 * (i + 1)]
                                          for i in range(4)], axis=1))
        o[f"b2_{l}"] = F(np.asarray(tp["b2"]).reshape(HID, 1))
    R4v = np.zeros((4, 128), np.float32)
    E4v = np.zeros((128, 4), np.float32)
    for j in range(4):
        R4v[j, 32 * j:32 * j + 32] = 1.0
        E4v[32 * j + 16, j] = 1.0
    o["R4c"] = BF(R4v)
    o["E4c"] = BF(E4v)
    o["gW1T"] = BF(np.asarray(p["gW1"]).T)
    o["gb1"] = F(np.asarray(p["gb1"]).reshape(HID, 1))
    o["gW2T"] = BF(np.asarray(p["gW2"]).T)
    o["gb2"] = F(np.asarray(p["gb2"]).reshape(1, 1))
    return o


def _make_runner(nc):
    """Cached jitted SPMD executor (replicates run_bass_via_pjrt but keeps
    the jitted callable so repeat calls skip compilation)."""
    import jax
    import numpy as np
    from jax.sharding import Mesh, PartitionSpec
    from jax.experimental.shard_map import shard_map
    import concourse.mybir as mybir
    from concourse import bass2jax

    bass2jax.install_neuronx_cc_hook()
    partition_name = (nc.partition_id_tensor.name
                      if nc.partition_id_tensor else None)
    in_names, out_names, out_avals, zero_outs = [], [], [], []
    for alloc in nc.m.functions[0].allocations:
        if not isinstance(alloc, mybir.MemoryLocationSet):
            continue
        name = alloc.memorylocations[0].name
        if alloc.kind == "ExternalInput":
            if name != partition_name:
                in_names.append(name)
        elif alloc.kind == "ExternalOutput":
            out_names.append(name)
            shape = tuple(alloc.tensor_shape)
            dtype = mybir.dt.np(alloc.dtype)
            out_avals.append(jax.core.ShapedArray(shape, dtype))
            zero_outs.append(np.zeros(shape, dtype))
    n_params = len(in_names)
    n_outs = len(out_avals)
    all_names = list(in_names) + list(out_names)
    if partition_name is not None:
        all_names.append(partition_name)
    donate = tuple(range(n_params, n_params + n_outs))

    def _body(*args):
        operands = list(args)
        if partition_name is not None:
            operands.append(bass2jax.partition_id_tensor())
        outs = bass2jax._bass_exec_p.bind(
            *operands, out_avals=tuple(out_avals), in_names=tuple(all_names),
            out_names=tuple(out_names), lowering_input_output_aliases=(),
            sim_require_finite=True, sim_require_nnan=True, nc=nc)
        return tuple(outs)

    devices = jax.devices()[:NCORES]
    mesh = Mesh(np.asarray(devices), ("core",))
    in_specs = (PartitionSpec("core"),) * (n_params + n_outs)
    out_specs = (PartitionSpec("core"),) * n_outs
    sharded = jax.jit(
        shard_map(_body, mesh=mesh, in_specs=in_specs, out_specs=out_specs,
                  check_rep=False),
        donate_argnums=donate, keep_unused=True)

    def run(in_maps):
        per_core = [[np.asarray(m[nm]) for nm in in_names] for m in in_maps]
        concat_in = [np.concatenate([per_core[c][i] for c in range(NCORES)],
                                    axis=0) for i in range(n_params)]
        zo = [np.concatenate([z] * NCORES, axis=0) for z in zero_outs]
        _CACHE["timing"] = (sharded, concat_in, zero_outs)
        outs = sharded(*concat_in, *zo)
        results = []
        for c in range(NCORES):
            res = {}
            for i, nm in enumerate(out_names):
                arr = np.asarray(outs[i])
                k = arr.shape[0] // NCORES
                res[nm] = arr[c * k:(c + 1) * k]
            results.append(res)
        return results

    return run


def kernel(x, edge_attr, params, edge_index, n_per_graph):
    if "nc" not in _CACHE:
        _CACHE["nc"] = _build()
        _CACHE["runner"] = _make_runner(_CACHE["nc"])
    nc = _CACHE["nc"]

    x = np.asarray(x, np.float32)
    ea = np.asarray(edge_attr, np.float32)[:, 0]
    ei = np.asarray(edge_index)
    pshared = _prep_params(params)

    in_maps = []
    for c in range(NCORES):
        nlo = c * NC
        emask = (ei[0] >= nlo) & (ei[0] < nlo + NC)
        ins = _prep_core(x[nlo:nlo + NC], ea[emask], ei[0, emask] - nlo,
                         ei[1, emask] - nlo)
        ins.update(pshared)
        m = {}
        for k, v in ins.items():
            if v.dtype == ml_dtypes.bfloat16:
                m[k] = np.ascontiguousarray(v).view(np.uint16)
            else:
                m[k] = np.ascontiguousarray(v)
        in_maps.append(m)

    results = _CACHE["runner"](in_maps)

    denom = 0.0
    xps, vns = [], []
    for c in range(NCORES):
        res = results[c]
        denom += float(res["gws_o"][0, 0])
        xps.append(np.asarray(res["xp_o"], np.float32))
        vns.append(np.asarray(res["vn_o"], np.float32))
    xp = np.concatenate(xps, axis=1).T / denom
    vnv = np.concatenate(vns, axis=1).T
    xcat = np.concatenate([xp, vnv], axis=1)
    h = np.maximum(xcat @ np.asarray(params["hW1"], np.float32).T
                   + np.asarray(params["hb1"], np.float32), 0.0)
    h = np.maximum(h @ np.asarray(params["hW2"], np.float32).T
                   + np.asarray(params["hb2"], np.float32), 0.0)
    out = h @ np.asarray(params["hW3"], np.float32).T \
        + np.asarray(params["hb3"], np.float32)
    return out[:, 0].astype(np.float32)
